# revision 9
# baseline (speedup 1.0000x reference)
"""Trainium2 Bass kernel for nn_GAT_FP (3-layer GAT message passing), 8 cores.

Sharding: nodes split 1250/core (dst-owner). Edges sorted by dst, grouped
into 10 windows of 128 consecutive owned dst rows per core. Per window one
dma_gather pulls all source-node feature rows; dst-side expansion and
segment-sum run as selection-matrix matmuls on the PE. Source feature
tables (fs / zes / fs1) are AllGathered after the dense projections.
Feature masks are folded into the weight matrices on the host; bias rows
are augmented into the contraction dim. Segment softmax skips the
max-subtraction (logits are O(1); exp cannot overflow). The wide dense
path (h, layer-0/1 weights, fs/fd tables) runs in bf16 with fp32 PSUM
accumulation; preprocessing, softmax and epilogues stay fp32.
"""
import sys
sys.path.insert(0, "/opt/trn_rl_repo")
import math
import numpy as np
import ml_dtypes

import concourse.bass as bass
import concourse.tile as tile
from concourse import bacc, mybir
from concourse.bass_utils import run_bass_kernel_spmd
from concourse.masks import make_identity

F32 = mybir.dt.float32
BF16 = mybir.dt.bfloat16
I16 = mybir.dt.int16
I32 = mybir.dt.int32
AF = mybir.ActivationFunctionType
OP = mybir.AluOpType
AX = mybir.AxisListType
NPBF = ml_dtypes.bfloat16

N, E, IN = 10000, 64000, 1247
H, D0, D1, OUT = 4, 256, 8, 6
HD0, HD1 = H * D0, H * D1          # 1024, 32
NC = 8
NPC = N // NC                       # 1250 nodes per core
WPC = (NPC + 127) // 128            # 10 windows per core
KA = IN + 1                         # 1248 augmented contraction dim
K0T = (KA + 127) // 128             # 10 k-tiles layer-0 dense
K1T = HD0 // 128                    # 8 k-tiles layer-1 dense
NEG = -30000.0                      # pad logit bias -> exp == 0

_compiled = {}
last_exec_ns = None
_last_in_maps = None


def _wrows(w):
    return min(128, NPC - w * 128)


def _build_program(Ts):
    totT = sum(Ts)
    Tmax = max(Ts)
    toff = [sum(Ts[:w]) for w in range(WPC)]
    nc = bacc.Bacc("TRN2", target_bir_lowering=False, debug=False,
                   num_devices=NC)

    feat = nc.dram_tensor("feat", [NPC, IN], F32, kind="ExternalInput")
    wl0a = nc.dram_tensor("wl0a", [KA, HD0], BF16, kind="ExternalInput")
    wr0a = nc.dram_tensor("wr0a", [KA, HD0], BF16, kind="ExternalInput")
    wres0a = nc.dram_tensor("wres0a", [KA, HD0], BF16, kind="ExternalInput")
    w2p = nc.dram_tensor("w2p", [KA, HD1], BF16, kind="ExternalInput")
    wl1 = nc.dram_tensor("wl1", [HD0, HD1], BF16, kind="ExternalInput")
    wr1 = nc.dram_tensor("wr1", [HD0, HD1], BF16, kind="ExternalInput")
    wres1 = nc.dram_tensor("wres1", [HD0, HD1], BF16, kind="ExternalInput")
    wlin = nc.dram_tensor("wlin", [2 * HD1, OUT], F32, kind="ExternalInput")
    a0bc = nc.dram_tensor("a0bc", [128, HD0], F32, kind="ExternalInput")
    a1bc = nc.dram_tensor("a1bc", [128, HD1], F32, kind="ExternalInput")
    a2sbc = nc.dram_tensor("a2sbc", [128, HD1], F32, kind="ExternalInput")
    a2dbc = nc.dram_tensor("a2dbc", [128, HD1], F32, kind="ExternalInput")
    b1bc = nc.dram_tensor("b1bc", [128, HD1], F32, kind="ExternalInput")
    blinbc = nc.dram_tensor("blinbc", [128, OUT], F32, kind="ExternalInput")
    srcidx = nc.dram_tensor("srcidx", [128, 8 * totT], I16, kind="ExternalInput")
    dstloc = nc.dram_tensor("dstloc", [128, totT], F32, kind="ExternalInput")
    vbias = nc.dram_tensor("vbias", [128, totT], F32, kind="ExternalInput")
    out_ext = nc.dram_tensor("out", [NPC, OUT], F32, kind="ExternalOutput")

    with tile.TileContext(nc) as tc:
        with tc.tile_pool(name="dram", bufs=1, space="DRAM") as dram, \
             tc.tile_pool(name="constp", bufs=1) as constp, \
             tc.tile_pool(name="hold", bufs=1) as hold, \
             tc.tile_pool(name="work", bufs=2) as work:

            fs_c = dram.tile([NPC, HD0], BF16)
            fd_c = dram.tile([NPC, HD0], BF16)
            res_c = dram.tile([NPC, HD0], F32)
            zes_c = dram.tile([NPC, 64], F32)
            fs1p_c = dram.tile([NPC, 64], F32)
            fs_full = dram.tile([N, HD0], BF16, addr_space="Shared")
            zes_full = dram.tile([N, 64], F32, addr_space="Shared")
            fs1p_full = dram.tile([N, 64], F32, addr_space="Shared")
            cs_bounce = dram.tile([1, IN], F32)
            cs_sum = dram.tile([1, IN], F32, addr_space="Shared")

            ident = constp.tile([128, 128], F32)
            make_identity(nc, ident[:])
            iota_row_i = constp.tile([128, 128], I32)
            nc.gpsimd.iota(iota_row_i[:], pattern=[[1, 128]], channel_multiplier=0)
            iota_row = constp.tile([128, 128], F32)
            nc.vector.tensor_copy(out=iota_row[:], in_=iota_row_i[:])
            iota_col_i = constp.tile([128, 128], I32)
            nc.gpsimd.iota(iota_col_i[:], pattern=[[0, 128]], channel_multiplier=1)
            iota_col = constp.tile([128, 128], F32)
            nc.vector.tensor_copy(out=iota_col[:], in_=iota_col_i[:])
            ones128 = constp.tile([128, 1], F32)
            nc.vector.memset(ones128[:], 1.0)
            ones_row = constp.tile([1, 128], F32)
            nc.vector.memset(ones_row[:], 1.0)

            def load_const(name, dramt, shape, dt=F32):
                t = constp.tile(shape, dt, tag=name, name=name)
                nc.sync.dma_start(out=t[:], in_=dramt[:])
                return t
            a0b = load_const("a0b", a0bc, [128, HD0])
            a1b = load_const("a1b", a1bc, [128, HD1])
            a2sb = load_const("a2sb", a2sbc, [128, HD1])
            a2db = load_const("a2db", a2dbc, [128, HD1])
            b1b = load_const("b1b", b1bc, [128, HD1])
            blinb = load_const("blinb", blinbc, [128, OUT])
            dlocs = load_const("dlocs", dstloc, [128, totT])
            vbs = load_const("vbs", vbias, [128, totT])
            sidx = load_const("sidx", srcidx, [128, 8 * totT], I16)
            wlsb = load_const("wlsb", wlin, [2 * HD1, OUT])

            catT = hold.tile([64, NPC], F32)
            ed_t = [hold.tile([128, H], BF16, tag=f"ed{m}", name=f"ed{m}")
                    for m in range(WPC)]
            fd1_t = [hold.tile([128, HD1], F32, tag=f"fd1_{m}", name=f"fd1_{m}")
                     for m in range(WPC)]
            res1_t = [hold.tile([128, HD1], F32, tag=f"res1_{m}",
                                name=f"res1_{m}") for m in range(WPC)]

            ncol = [(j * 512, min(512, IN - j * 512))
                    for j in range((IN + 511) // 512)]

            # ============ Phases P, D0, z (hT alive) ============
            with tc.tile_pool(name="hpool", bufs=1) as hpool:
                hT = hpool.tile([128, K0T * NPC], BF16)

                with tc.tile_pool(name="pp", bufs=1) as pp, \
                     tc.tile_pool(name="psP", bufs=1, space="PSUM") as psP:
                    # pass 1: column sums (streamed feature tiles)
                    cs_sb = pp.tile([1, IN], F32, tag="cs_sb")
                    cpss = [psP.tile([1, 512], F32, tag=f"cs{j}", name=f"cs{j}",
                                     space="PSUM") for j in range(len(ncol))]
                    for m in range(WPC):
                        pr = _wrows(m)
                        ft = pp.tile([128, IN], F32, tag="fstream", name="ft",
                                     bufs=3)
                        nc.sync.dma_start(out=ft[:pr, :],
                                          in_=feat[m * 128:m * 128 + pr, :])
                        for j, (c0, cw) in enumerate(ncol):
                            nc.tensor.matmul(out=cpss[j][:1, :cw],
                                             lhsT=ones128[:pr, :],
                                             rhs=ft[:pr, c0:c0 + cw],
                                             start=(m == 0), stop=(m == WPC - 1))
                    for j, (c0, cw) in enumerate(ncol):
                        nc.scalar.copy(out=cs_sb[:, c0:c0 + cw],
                                       in_=cpss[j][:1, :cw])
                    nc.gpsimd.dma_start(out=cs_bounce[:], in_=cs_sb[:])
                    nc.gpsimd.collective_compute(
                        "AllReduce", OP.add, replica_groups=[list(range(NC))],
                        ins=[cs_bounce[:]], outs=[cs_sum[:]])
                    meanh = pp.tile([1, IN], F32, tag="meanh")
                    nc.sync.dma_start(out=meanh[:], in_=cs_sum[:])
                    nc.scalar.mul(out=meanh[:], in_=meanh[:], mul=0.5 / N)
                    meanb = pp.tile([128, IN], F32, tag="meanb")
                    for j, (c0, cw) in enumerate(ncol):
                        bps = psP.tile([128, 512], F32, tag="bps", name="bps",
                                       space="PSUM")
                        nc.tensor.matmul(out=bps[:, :cw], lhsT=ones_row[:, :],
                                         rhs=meanh[:, c0:c0 + cw],
                                         start=True, stop=True)
                        nc.scalar.copy(out=meanb[:, c0:c0 + cw], in_=bps[:, :cw])

                    # pass 2: impute + L1-normalize + transpose into hT (bf16)
                    for m in range(WPC):
                        pr = _wrows(m)
                        ft = pp.tile([128, KA], F32, tag="fstream2", name="ft",
                                     bufs=3)
                        nc.sync.dma_start(out=ft[:pr, 0:IN],
                                          in_=feat[m * 128:m * 128 + pr, :])
                        nc.vector.memset(ft[:, IN:KA], 1.0)
                        msk = pp.tile([128, IN], F32, tag="msk", name="msk",
                                      bufs=2)
                        nc.vector.tensor_scalar(out=msk[:pr, :],
                                                in0=ft[:pr, 0:IN],
                                                scalar1=0.0, scalar2=None,
                                                op0=OP.is_equal)
                        nc.vector.tensor_tensor(out=msk[:pr, :], in0=msk[:pr, :],
                                                in1=meanb[:pr, :], op=OP.mult)
                        nc.vector.tensor_tensor(out=ft[:pr, 0:IN],
                                                in0=ft[:pr, 0:IN],
                                                in1=msk[:pr, :], op=OP.add)
                        rs = work.tile([128, 1], F32, tag="rs")
                        nc.vector.tensor_reduce(out=rs[:pr, :],
                                                in_=ft[:pr, 0:IN],
                                                axis=AX.X, op=OP.add,
                                                apply_absolute_value=True)
                        nc.vector.tensor_scalar(out=rs[:pr, :], in0=rs[:pr, :],
                                                scalar1=1e-12, scalar2=None,
                                                op0=OP.max)
                        rinv = work.tile([128, 1], F32, tag="rinv")
                        nc.vector.reciprocal(out=rinv[:pr, :], in_=rs[:pr, :])
                        nc.vector.tensor_scalar(out=ft[:pr, 0:IN],
                                                in0=ft[:pr, 0:IN],
                                                scalar1=rinv[:pr, 0:1],
                                                scalar2=None, op0=OP.mult)
                        for k in range(K0T):
                            kw = min(128, KA - k * 128)
                            tps = psP.tile([128, 128], F32, tag="tps", name="tps",
                                           space="PSUM", bufs=2)
                            nc.tensor.transpose(out=tps[:kw, :pr],
                                                in_=ft[:pr, k * 128:k * 128 + kw],
                                                identity=ident[:pr, :pr])
                            nc.scalar.copy(
                                out=hT[:kw, k * NPC + m * 128:
                                       k * NPC + m * 128 + pr],
                                in_=tps[:kw, :pr])

                # ---------- D0 dense (bf16 x bf16 -> f32 psum) ----------
                with tc.tile_pool(name="dpool", bufs=1) as dpool, \
                     tc.tile_pool(name="psD", bufs=2, space="PSUM") as psD:

                    def dense0(wdram, dest, odt):
                        wkt = [dpool.tile([128, HD0], BF16, tag=f"wk{k}",
                                          name=f"wk{k}", bufs=2)
                               for k in range(K0T)]
                        for k in range(K0T):
                            kw = min(128, KA - k * 128)
                            nc.sync.dma_start(out=wkt[k][:kw, :],
                                              in_=wdram[k * 128:k * 128 + kw, :])
                        for m in range(WPC):
                            pr = _wrows(m)
                            osb = dpool.tile([128, HD0], odt, tag=f"d0o{odt}",
                                             name="d0o", bufs=2)
                            for j in range(2):
                                ops = psD.tile([128, 512], F32, tag="d0ps",
                                               name="d0ps", space="PSUM")
                                for k in range(K0T):
                                    kw = min(128, KA - k * 128)
                                    nc.tensor.matmul(
                                        out=ops[:pr, :],
                                        lhsT=hT[:kw, k * NPC + m * 128:
                                                k * NPC + m * 128 + pr],
                                        rhs=wkt[k][:kw, j * 512:(j + 1) * 512],
                                        start=(k == 0), stop=(k == K0T - 1))
                                nc.scalar.copy(
                                    out=osb[:pr, j * 512:(j + 1) * 512],
                                    in_=ops[:pr, :])
                            nc.sync.dma_start(out=dest[m * 128:m * 128 + pr, :],
                                              in_=osb[:pr, :])

                    dense0(wl0a, fs_c, BF16)
                    nc.gpsimd.collective_compute(
                        "AllGather", OP.bypass, replica_groups=[list(range(NC))],
                        ins=[fs_c[:]], outs=[fs_full[:]])
                    dense0(wr0a, fd_c, BF16)
                    dense0(wres0a, res_c, F32)

                    w2sb = dpool.tile([128, K0T * HD1], BF16, tag="w2sb")
                    for k in range(K0T):
                        kw = min(128, KA - k * 128)
                        nc.sync.dma_start(out=w2sb[:kw, k * HD1:(k + 1) * HD1],
                                          in_=w2p[k * 128:k * 128 + kw, :])
                    for m in range(WPC):
                        pr = _wrows(m)
                        zps = psD.tile([128, HD1], F32, tag="zps", name="zps",
                                       space="PSUM")
                        for k in range(K0T):
                            kw = min(128, KA - k * 128)
                            nc.tensor.matmul(
                                out=zps[:pr, :],
                                lhsT=hT[:kw, k * NPC + m * 128:
                                        k * NPC + m * 128 + pr],
                                rhs=w2sb[:kw, k * HD1:(k + 1) * HD1],
                                start=(k == 0), stop=(k == K0T - 1))
                        zsb = dpool.tile([128, 64], F32, tag="zsb", name="zsb",
                                         bufs=2)
                        nc.vector.memset(zsb[:], 0.0)
                        nc.scalar.copy(out=zsb[:pr, 0:HD1], in_=zps[:pr, :])
                        tmp = dpool.tile([128, HD1], F32, tag="ztmp", name="ztmp",
                                         bufs=2)
                        nc.vector.tensor_tensor(out=tmp[:pr, :],
                                                in0=zsb[:pr, 0:HD1],
                                                in1=a2sb[:pr, :], op=OP.mult)
                        nc.vector.tensor_reduce(
                            out=zsb[:pr, 32:36],
                            in_=tmp[:pr, :].rearrange("p (h d) -> p h d", h=H),
                            axis=AX.X, op=OP.add)
                        edf = dpool.tile([128, H], F32, tag="edf", name="edf",
                                         bufs=2)
                        nc.vector.tensor_tensor(out=tmp[:pr, :],
                                                in0=zsb[:pr, 0:HD1],
                                                in1=a2db[:pr, :], op=OP.mult)
                        nc.vector.tensor_reduce(
                            out=edf[:pr, :],
                            in_=tmp[:pr, :].rearrange("p (h d) -> p h d", h=H),
                            axis=AX.X, op=OP.add)
                        nc.vector.tensor_copy(out=ed_t[m][:pr, :],
                                              in_=edf[:pr, :])
                        nc.sync.dma_start(out=zes_c[m * 128:m * 128 + pr, :],
                                          in_=zsb[:pr, :])
                    nc.gpsimd.collective_compute(
                        "AllGather", OP.bypass, replica_groups=[list(range(NC))],
                        ins=[zes_c[:]], outs=[zes_full[:]])

            # ============ shared sel-matrix builder ============
            def build_sel(t, dl, pspool, e0mode):
                sel = work.tile([128, 128], F32, tag=f"sel{t}", name=f"sel{t}")
                nc.vector.tensor_tensor(out=sel[:],
                                        in0=dl.to_broadcast([128, 128]),
                                        in1=iota_row[:], op=OP.is_equal)
                dps = pspool.tile([128, 128], F32, tag="dps", name="dps",
                                  space="PSUM")
                nc.tensor.transpose(out=dps[:], in_=dl.to_broadcast([128, 128]),
                                    identity=ident[:])
                dlT = work.tile([128, 128], F32, tag="dlT")
                nc.scalar.copy(out=dlT[:], in_=dps[:])
                if e0mode:
                    selT16 = work.tile([128, 128], BF16, tag="selT16",
                                       name="selT16")
                    nc.vector.tensor_tensor(out=selT16[:], in0=iota_col[:],
                                            in1=dlT[:], op=OP.is_equal)
                    sel16 = work.tile([128, 128], BF16, tag=f"sel16_{t}",
                                      name=f"sel16_{t}")
                    nc.vector.tensor_copy(out=sel16[:], in_=sel[:])
                    return sel, sel16, selT16
                selT = work.tile([128, 128], F32, tag="selT", name="selT")
                nc.vector.tensor_tensor(out=selT[:], in0=iota_col[:], in1=dlT[:],
                                        op=OP.is_equal)
                return sel, selT

            # ============ Phase E0 + inner (h1T alive through D1) ============
            with tc.tile_pool(name="h1pool", bufs=1) as h1pool:
                h1T = h1pool.tile([128, K1T * NPC], BF16)
                with tc.tile_pool(name="psE0", bufs=1, space="PSUM") as psE0:
                    for w in range(WPC):
                        T = Ts[w]
                        nloc = _wrows(w)
                        co = toff[w]
                        fsg = work.tile([128, Tmax * HD0], BF16, tag="fsg")
                        nc.gpsimd.dma_gather(
                            out_ap=fsg[:].rearrange("p (t e) -> p t e",
                                                    t=Tmax)[:, :T, :],
                            in_ap=fs_full[:],
                            idxs_ap=sidx[:, 8 * co:8 * (co + T)],
                            num_idxs=T * 128, num_idxs_reg=T * 128,
                            elem_size=HD0)
                        zesg = work.tile([128, Tmax * 64], F32, tag="zesg")
                        nc.gpsimd.dma_gather(
                            out_ap=zesg[:].rearrange("p (t e) -> p t e",
                                                     t=Tmax)[:, :T, :],
                            in_ap=zes_full[:],
                            idxs_ap=sidx[:, 8 * co:8 * (co + T)],
                            num_idxs=T * 128, num_idxs_reg=T * 128,
                            elem_size=64)
                        fdw = work.tile([128, HD0], BF16, tag="fdw")
                        nc.sync.dma_start(out=fdw[:nloc, :],
                                          in_=fd_c[w * 128:w * 128 + nloc, :])
                        resw = work.tile([128, HD0], F32, tag="resw")
                        nc.sync.dma_start(out=resw[:nloc, :],
                                          in_=res_c[w * 128:w * 128 + nloc, :])

                        sels, sel16s, els, el2s = [], [], [], []
                        dn_ps = psE0.tile([128, H], F32, tag="dn", name="dn",
                                          space="PSUM")
                        dn2_ps = psE0.tile([128, H], F32, tag="dn2", name="dn2",
                                           space="PSUM")
                        for t in range(T):
                            dl = dlocs[:, co + t:co + t + 1]
                            vb = vbs[:, co + t:co + t + 1]
                            sel, sel16, selT16 = build_sel(t, dl, psE0, True)
                            sels.append(sel)
                            sel16s.append(sel16)
                            fdx = psE0.tile([128, HD0], F32, tag="mm1024",
                                            name="fdx", space="PSUM")
                            for j in range(2):
                                nc.tensor.matmul(
                                    out=fdx[:, j * 512:(j + 1) * 512],
                                    lhsT=selT16[:nloc, :],
                                    rhs=fdw[:nloc, j * 512:(j + 1) * 512],
                                    start=True, stop=True)
                            tt = work.tile([128, HD0], F32, tag="tt")
                            nc.vector.tensor_tensor(
                                out=tt[:], in0=fsg[:, t * HD0:(t + 1) * HD0],
                                in1=fdx[:], op=OP.add)
                            nc.scalar.activation(out=tt[:], in_=tt[:],
                                                 func=AF.Lrelu, alpha=0.2)
                            nc.vector.tensor_tensor(out=tt[:], in0=tt[:],
                                                    in1=a0b[:], op=OP.mult)
                            lg = work.tile([128, H], F32, tag="lg")
                            nc.vector.tensor_reduce(
                                out=lg[:],
                                in_=tt[:].rearrange("p (h d) -> p h d", h=H),
                                axis=AX.X, op=OP.add)
                            el = work.tile([128, H], F32, tag=f"el{t}",
                                           name=f"el{t}")
                            nc.scalar.activation(out=el[:], in_=lg[:],
                                                 func=AF.Exp, bias=vb)
                            els.append(el)
                            nc.tensor.matmul(out=dn_ps[:], lhsT=sel[:, :],
                                             rhs=el[:], start=(t == 0),
                                             stop=(t == T - 1))
                            edx = psE0.tile([128, HD1], F32, tag="m32",
                                            name="edx", space="PSUM")
                            nc.tensor.matmul(out=edx[:, 0:H],
                                             lhsT=selT16[:nloc, :],
                                             rhs=ed_t[w][:nloc, :],
                                             start=True, stop=True)
                            lg2 = work.tile([128, H], F32, tag="lg2")
                            nc.vector.tensor_tensor(
                                out=lg2[:],
                                in0=zesg[:, t * 64 + 32:t * 64 + 36],
                                in1=edx[:, 0:H], op=OP.add)
                            nc.scalar.activation(out=lg2[:], in_=lg2[:],
                                                 func=AF.Lrelu, alpha=0.2)
                            el2 = work.tile([128, H], F32, tag=f"el2_{t}",
                                            name=f"el2_{t}")
                            nc.scalar.activation(out=el2[:], in_=lg2[:],
                                                 func=AF.Exp, bias=vb)
                            el2s.append(el2)
                            nc.tensor.matmul(out=dn2_ps[:], lhsT=sel[:, :],
                                             rhs=el2[:], start=(t == 0),
                                             stop=(t == T - 1))

                        idn = work.tile([128, H], F32, tag="idn")
                        nc.vector.tensor_scalar(out=idn[:], in0=dn_ps[:],
                                                scalar1=1e-9, scalar2=None,
                                                op0=OP.max)
                        nc.vector.reciprocal(out=idn[:], in_=idn[:])
                        idn2 = work.tile([128, H], F32, tag="idn2")
                        nc.vector.tensor_scalar(out=idn2[:], in0=dn2_ps[:],
                                                scalar1=1e-9, scalar2=None,
                                                op0=OP.max)
                        nc.vector.reciprocal(out=idn2[:], in_=idn2[:])

                        o_ps = psE0.tile([128, HD0], F32, tag="mm1024",
                                         name="o_ps", space="PSUM")
                        oz_ps = psE0.tile([128, HD1], F32, tag="m32",
                                          name="oz_ps", space="PSUM")
                        for t in range(T):
                            for h in range(H):
                                nc.vector.tensor_scalar(
                                    out=fsg[:, t * HD0 + h * D0:
                                            t * HD0 + (h + 1) * D0],
                                    in0=fsg[:, t * HD0 + h * D0:
                                            t * HD0 + (h + 1) * D0],
                                    scalar1=els[t][:, h:h + 1], scalar2=None,
                                    op0=OP.mult)
                                nc.vector.tensor_scalar(
                                    out=zesg[:, t * 64 + h * D1:
                                             t * 64 + (h + 1) * D1],
                                    in0=zesg[:, t * 64 + h * D1:
                                             t * 64 + (h + 1) * D1],
                                    scalar1=el2s[t][:, h:h + 1], scalar2=None,
                                    op0=OP.mult)
                            for j in range(2):
                                nc.tensor.matmul(
                                    out=o_ps[:, j * 512:(j + 1) * 512],
                                    lhsT=sel16s[t][:, :],
                                    rhs=fsg[:, t * HD0 + j * 512:
                                            t * HD0 + (j + 1) * 512],
                                    start=(t == 0), stop=(t == T - 1))
                            nc.tensor.matmul(out=oz_ps[:], lhsT=sels[t][:, :],
                                             rhs=zesg[:, t * 64:t * 64 + HD1],
                                             start=(t == 0), stop=(t == T - 1))

                        ho = work.tile([128, HD0], F32, tag="ho")
                        for h in range(H):
                            nc.vector.tensor_scalar(
                                out=ho[:nloc, h * D0:(h + 1) * D0],
                                in0=o_ps[:nloc, h * D0:(h + 1) * D0],
                                scalar1=idn[:nloc, h:h + 1], scalar2=None,
                                op0=OP.mult)
                        nc.vector.tensor_tensor(out=ho[:nloc, :],
                                                in0=ho[:nloc, :],
                                                in1=resw[:nloc, :], op=OP.add)
                        nc.scalar.activation(out=ho[:nloc, :], in_=ho[:nloc, :],
                                             func=AF.Relu)
                        for k in range(K1T):
                            tps = psE0.tile([128, 128], F32, tag="tpsE",
                                            name="tpsE", space="PSUM")
                            nc.tensor.transpose(
                                out=tps[:, :nloc],
                                in_=ho[:nloc, k * 128:(k + 1) * 128],
                                identity=ident[:nloc, :nloc])
                            nc.scalar.copy(
                                out=h1T[:, k * NPC + w * 128:
                                        k * NPC + w * 128 + nloc],
                                in_=tps[:, :nloc])
                        ozs = work.tile([128, HD1], F32, tag="ozs")
                        for h in range(H):
                            nc.vector.tensor_scalar(
                                out=ozs[:nloc, h * D1:(h + 1) * D1],
                                in0=oz_ps[:nloc, h * D1:(h + 1) * D1],
                                scalar1=idn2[:nloc, h:h + 1], scalar2=None,
                                op0=OP.mult)
                        zt = psE0.tile([128, 128], F32, tag="tpsE", name="zt",
                                       space="PSUM")
                        nc.tensor.transpose(out=zt[:HD1, :nloc],
                                            in_=ozs[:nloc, :],
                                            identity=ident[:nloc, :nloc])
                        nc.scalar.copy(out=catT[0:HD1, w * 128:w * 128 + nloc],
                                       in_=zt[:HD1, :nloc])

                # ---------- D1 dense ----------
                with tc.tile_pool(name="d1pool", bufs=1) as d1pool, \
                     tc.tile_pool(name="psD1", bufs=2, space="PSUM") as psD1:
                    wk1 = {}
                    for nm, wd in (("l", wl1), ("r", wr1), ("res", wres1)):
                        t = d1pool.tile([128, K1T * HD1], BF16, tag=f"wk1{nm}",
                                        name=f"wk1{nm}")
                        for k in range(K1T):
                            nc.sync.dma_start(out=t[:, k * HD1:(k + 1) * HD1],
                                              in_=wd[k * 128:(k + 1) * 128, :])
                        wk1[nm] = t
                    for m in range(WPC):
                        pr = _wrows(m)
                        outs = {}
                        for nm in ("l", "r", "res"):
                            p1 = psD1.tile([128, HD1], F32, tag=f"d1{nm}",
                                           name=f"d1{nm}", space="PSUM")
                            for k in range(K1T):
                                nc.tensor.matmul(
                                    out=p1[:pr, :],
                                    lhsT=h1T[:, k * NPC + m * 128:
                                             k * NPC + m * 128 + pr],
                                    rhs=wk1[nm][:, k * HD1:(k + 1) * HD1],
                                    start=(k == 0), stop=(k == K1T - 1))
                            outs[nm] = p1
                        f1 = d1pool.tile([128, 64], F32, tag="f1", name="f1",
                                         bufs=2)
                        nc.vector.memset(f1[:], 0.0)
                        nc.scalar.copy(out=f1[:pr, 0:HD1], in_=outs["l"][:pr, :])
                        nc.sync.dma_start(out=fs1p_c[m * 128:m * 128 + pr, :],
                                          in_=f1[:pr, :])
                        nc.scalar.copy(out=fd1_t[m][:pr, :],
                                       in_=outs["r"][:pr, :])
                        nc.scalar.copy(out=res1_t[m][:pr, :],
                                       in_=outs["res"][:pr, :])
                    nc.gpsimd.collective_compute(
                        "AllGather", OP.bypass, replica_groups=[list(range(NC))],
                        ins=[fs1p_c[:]], outs=[fs1p_full[:]])

            # ============ Phase E1 (all f32) ============
            with tc.tile_pool(name="psE1", bufs=1, space="PSUM") as psE1:
                for w in range(WPC):
                    T = Ts[w]
                    nloc = _wrows(w)
                    co = toff[w]
                    f1g = work.tile([128, Tmax * 64], F32, tag="f1g")
                    nc.gpsimd.dma_gather(
                        out_ap=f1g[:].rearrange("p (t e) -> p t e",
                                                t=Tmax)[:, :T, :],
                        in_ap=fs1p_full[:],
                        idxs_ap=sidx[:, 8 * co:8 * (co + T)],
                        num_idxs=T * 128, num_idxs_reg=T * 128, elem_size=64)
                    sels, els = [], []
                    dn_ps = psE1.tile([128, H], F32, tag="dn3", name="dn3",
                                      space="PSUM")
                    for t in range(T):
                        dl = dlocs[:, co + t:co + t + 1]
                        vb = vbs[:, co + t:co + t + 1]
                        sel, selT = build_sel(t, dl, psE1, False)
                        sels.append(sel)
                        fdx = psE1.tile([128, HD1], F32, tag="m32b", name="fdx1",
                                        space="PSUM")
                        nc.tensor.matmul(out=fdx[:], lhsT=selT[:nloc, :],
                                         rhs=fd1_t[w][:nloc, :], start=True,
                                         stop=True)
                        tt = work.tile([128, HD1], F32, tag="tt1")
                        nc.vector.tensor_tensor(out=tt[:],
                                                in0=f1g[:, t * 64:t * 64 + HD1],
                                                in1=fdx[:], op=OP.add)
                        nc.scalar.activation(out=tt[:], in_=tt[:], func=AF.Lrelu,
                                             alpha=0.2)
                        nc.vector.tensor_tensor(out=tt[:], in0=tt[:], in1=a1b[:],
                                                op=OP.mult)
                        lg = work.tile([128, H], F32, tag="lg3")
                        nc.vector.tensor_reduce(
                            out=lg[:],
                            in_=tt[:].rearrange("p (h d) -> p h d", h=H),
                            axis=AX.X, op=OP.add)
                        el = work.tile([128, H], F32, tag=f"el3_{t}",
                                       name=f"el3_{t}")
                        nc.scalar.activation(out=el[:], in_=lg[:], func=AF.Exp,
                                             bias=vb)
                        els.append(el)
                        nc.tensor.matmul(out=dn_ps[:], lhsT=sel[:, :], rhs=el[:],
                                         start=(t == 0), stop=(t == T - 1))
                    idn = work.tile([128, H], F32, tag="idn3")
                    nc.vector.tensor_scalar(out=idn[:], in0=dn_ps[:],
                                            scalar1=1e-9, scalar2=None,
                                            op0=OP.max)
                    nc.vector.reciprocal(out=idn[:], in_=idn[:])
                    o_ps = psE1.tile([128, HD1], F32, tag="m32b", name="o_ps1",
                                     space="PSUM")
                    for t in range(T):
                        for h in range(H):
                            nc.vector.tensor_scalar(
                                out=f1g[:, t * 64 + h * D1:
                                        t * 64 + (h + 1) * D1],
                                in0=f1g[:, t * 64 + h * D1:
                                        t * 64 + (h + 1) * D1],
                                scalar1=els[t][:, h:h + 1], scalar2=None,
                                op0=OP.mult)
                        nc.tensor.matmul(out=o_ps[:], lhsT=sels[t][:, :],
                                         rhs=f1g[:, t * 64:t * 64 + HD1],
                                         start=(t == 0), stop=(t == T - 1))
                    oo = work.tile([128, HD1], F32, tag="oo")
                    for h in range(H):
                        nc.vector.tensor_scalar(
                            out=oo[:nloc, h * D1:(h + 1) * D1],
                            in0=o_ps[:nloc, h * D1:(h + 1) * D1],
                            scalar1=idn[:nloc, h:h + 1], scalar2=None,
                            op0=OP.mult)
                    nc.vector.tensor_tensor(out=oo[:nloc, :], in0=oo[:nloc, :],
                                            in1=res1_t[w][:nloc, :], op=OP.add)
                    nc.vector.tensor_tensor(out=oo[:nloc, :], in0=oo[:nloc, :],
                                            in1=b1b[:nloc, :], op=OP.add)
                    nc.scalar.activation(out=oo[:nloc, :], in_=oo[:nloc, :],
                                         func=AF.Relu)
                    tp = psE1.tile([128, 128], F32, tag="tp1", name="tp1",
                                   space="PSUM")
                    nc.tensor.transpose(out=tp[:HD1, :nloc], in_=oo[:nloc, :],
                                        identity=ident[:nloc, :nloc])
                    nc.scalar.copy(out=catT[HD1:2 * HD1, w * 128:w * 128 + nloc],
                                   in_=tp[:HD1, :nloc])

            # ============ Phase F ============
            with tc.tile_pool(name="psF", bufs=2, space="PSUM") as psF:
                for m in range(WPC):
                    pr = _wrows(m)
                    fp = psF.tile([128, OUT], F32, tag="fin", name="fin",
                                  space="PSUM")
                    nc.tensor.matmul(out=fp[:pr, :],
                                     lhsT=catT[:, m * 128:m * 128 + pr],
                                     rhs=wlsb[:], start=True, stop=True)
                    osb = work.tile([128, OUT], F32, tag="osb")
                    nc.vector.tensor_tensor(out=osb[:pr, :], in0=fp[:pr, :],
                                            in1=blinb[:pr, :], op=OP.add)
                    nc.sync.dma_start(out=out_ext[m * 128:m * 128 + pr, :],
                                      in_=osb[:pr, :])

    nc.compile()
    return nc


def _prep_edges(src, dst):
    order = np.argsort(dst, kind="stable")
    ss = src[order].astype(np.int64)
    ds = dst[order].astype(np.int64)
    cnt = np.zeros((NC, WPC), np.int64)
    bounds = {}
    for c in range(NC):
        for w in range(WPC):
            lo = c * NPC + w * 128
            hi = min(c * NPC + (w + 1) * 128, (c + 1) * NPC)
            e0 = np.searchsorted(ds, lo, side="left")
            e1 = np.searchsorted(ds, hi, side="left")
            cnt[c, w] = e1 - e0
            bounds[(c, w)] = (e0, e1)
    nws = [int(cnt[:, w].max()) for w in range(WPC)]
    Ts = [max(1, math.ceil(nv / 128)) for nv in nws]
    totT = sum(Ts)
    per_core = []
    for c in range(NC):
        sidx = np.zeros((128, 8 * totT), np.int16)
        dloc = np.zeros((128, totT), np.float32)
        vb = np.full((128, totT), NEG, np.float32)
        co = 0
        for w in range(WPC):
            T = Ts[w]
            e0, e1 = bounds[(c, w)]
            k = e1 - e0
            slots = T * 128
            s = np.zeros(slots, np.int16)
            d = np.zeros(slots, np.float32)
            v = np.full(slots, NEG, np.float32)
            s[:k] = ss[e0:e1]
            d[:k] = (ds[e0:e1] - (c * NPC + w * 128)).astype(np.float32)
            v[:k] = 0.0
            cols = s.reshape(8 * T, 16).T
            sidx[:, 8 * co:8 * (co + T)] = np.tile(cols, (8, 1))
            dloc[:, co:co + T] = d.reshape(T, 128).T
            vb[:, co:co + T] = v.reshape(T, 128).T
            co += T
        per_core.append((sidx, dloc, vb))
    return Ts, per_core


def kernel(features, src, dst, textMask, audioMask, videoMask, W2, a2,
           Wl0, Wr0, a0, Wres0, b0, Wl1, Wr1, a1, Wres1, b1, Wlin, blin):
    features = np.asarray(features, np.float32)
    src = np.asarray(src, np.int32)
    dst = np.asarray(dst, np.int32)

    Ts, per_core = _prep_edges(src, dst)
    key = tuple(Ts)
    if key not in _compiled:
        _compiled.clear()
        _compiled[key] = _build_program(Ts)
    nc = _compiled[key]

    maskSum = (np.asarray(textMask) + np.asarray(audioMask)
               + np.asarray(videoMask)).astype(np.float32)

    def aug(Wm, brow=None):
        o = np.zeros((KA, Wm.shape[1]), np.float32)
        o[:IN] = Wm * maskSum[:, None]
        if brow is not None:
            o[IN] = brow
        return o.astype(NPBF)

    w2flat = np.asarray(W2, np.float32).transpose(1, 0, 2).reshape(IN, HD1)
    shared = {
        "wl0a": aug(np.asarray(Wl0, np.float32)),
        "wr0a": aug(np.asarray(Wr0, np.float32)),
        "wres0a": aug(np.asarray(Wres0, np.float32),
                      np.asarray(b0, np.float32)),
        "w2p": aug(w2flat),
        "wl1": np.asarray(Wl1, np.float32).astype(NPBF),
        "wr1": np.asarray(Wr1, np.float32).astype(NPBF),
        "wres1": np.asarray(Wres1, np.float32).astype(NPBF),
        "wlin": np.asarray(Wlin, np.float32),
        "a0bc": np.tile(np.asarray(a0, np.float32).reshape(1, HD0), (128, 1)),
        "a1bc": np.tile(np.asarray(a1, np.float32).reshape(1, HD1), (128, 1)),
        "a2sbc": np.tile(np.asarray(a2, np.float32)[:, :D1].reshape(1, HD1),
                         (128, 1)),
        "a2dbc": np.tile(np.asarray(a2, np.float32)[:, D1:].reshape(1, HD1),
                         (128, 1)),
        "b1bc": np.tile(np.asarray(b1, np.float32).reshape(1, HD1), (128, 1)),
        "blinbc": np.tile(np.asarray(blin, np.float32).reshape(1, OUT),
                          (128, 1)),
    }
    in_maps = []
    for c in range(NC):
        sidx, dloc, vb = per_core[c]
        m = dict(shared)
        m["feat"] = np.ascontiguousarray(features[c * NPC:(c + 1) * NPC])
        m["srcidx"] = sidx
        m["dstloc"] = dloc
        m["vbias"] = vb
        in_maps.append(m)

    global _last_in_maps
    _last_in_maps = in_maps
    res = run_bass_kernel_spmd(nc, in_maps, list(range(NC)))
    global last_exec_ns
    last_exec_ns = getattr(res, "exec_time_ns", None)
    return np.concatenate(
        [np.asarray(res.results[c]["out"]) for c in range(NC)], axis=0)


# revision 12
# speedup vs baseline: 16.8870x; 16.8870x over previous
"""Trainium2 Bass kernel for nn_GAT_FP (3-layer GAT message passing), 8 cores.

Sharding: nodes split 1250/core (dst-owner). Edges sorted by dst, grouped
into 10 windows of 128 consecutive owned dst rows per core. Per window one
dma_gather pulls all source-node feature rows; dst-side expansion and
segment-sum run as selection-matrix matmuls on the PE. Source feature
tables (fs / zes / fs1) are AllGathered after the dense projections.
Feature masks are folded into the weight matrices on the host; bias rows
are augmented into the contraction dim. Segment softmax skips the
max-subtraction (logits are O(1); exp cannot overflow). The wide dense
path (h, layer-0/1 weights, fs/fd tables) runs in bf16 with fp32 PSUM
accumulation; preprocessing, softmax and epilogues stay fp32.
"""
import sys
sys.path.insert(0, "/opt/trn_rl_repo")
import math
import numpy as np
import ml_dtypes

import concourse.bass as bass
import concourse.tile as tile
from concourse import bacc, mybir
from concourse.bass_utils import run_bass_kernel_spmd
from concourse.masks import make_identity

F32 = mybir.dt.float32
BF16 = mybir.dt.bfloat16
I16 = mybir.dt.int16
I32 = mybir.dt.int32
AF = mybir.ActivationFunctionType
OP = mybir.AluOpType
AX = mybir.AxisListType
NPBF = ml_dtypes.bfloat16

N, E, IN = 10000, 64000, 1247
H, D0, D1, OUT = 4, 256, 8, 6
HD0, HD1 = H * D0, H * D1          # 1024, 32
NC = 8
NPC = N // NC                       # 1250 nodes per core
WPC = (NPC + 127) // 128            # 10 windows per core
KA = IN + 1                         # 1248 augmented contraction dim
K0T = (KA + 127) // 128             # 10 k-tiles layer-0 dense
K1T = HD0 // 128                    # 8 k-tiles layer-1 dense
NEG = -30000.0                      # pad logit bias -> exp == 0

_compiled = {}
last_exec_ns = None
_last_in_maps = None


def _wrows(w):
    return min(128, NPC - w * 128)


def _build_program(Ts, reps=1, comms=True):
    totT = sum(Ts)
    Tmax = max(Ts)
    toff = [sum(Ts[:w]) for w in range(WPC)]
    nc = bacc.Bacc("TRN2", target_bir_lowering=False, debug=False,
                   num_devices=NC)

    feat = nc.dram_tensor("feat", [NPC, IN], F32, kind="ExternalInput")
    wl0a = nc.dram_tensor("wl0a", [KA, HD0], BF16, kind="ExternalInput")
    wr0a = nc.dram_tensor("wr0a", [KA, HD0], BF16, kind="ExternalInput")
    wres0a = nc.dram_tensor("wres0a", [KA, HD0], BF16, kind="ExternalInput")
    w2p = nc.dram_tensor("w2p", [KA, HD1], BF16, kind="ExternalInput")
    wl1 = nc.dram_tensor("wl1", [HD0, HD1], BF16, kind="ExternalInput")
    wr1 = nc.dram_tensor("wr1", [HD0, HD1], BF16, kind="ExternalInput")
    wres1 = nc.dram_tensor("wres1", [HD0, HD1], BF16, kind="ExternalInput")
    wlin = nc.dram_tensor("wlin", [2 * HD1, OUT], F32, kind="ExternalInput")
    a0bc = nc.dram_tensor("a0bc", [128, HD0], F32, kind="ExternalInput")
    a1bc = nc.dram_tensor("a1bc", [128, HD1], F32, kind="ExternalInput")
    a2sbc = nc.dram_tensor("a2sbc", [128, HD1], F32, kind="ExternalInput")
    a2dbc = nc.dram_tensor("a2dbc", [128, HD1], F32, kind="ExternalInput")
    b1bc = nc.dram_tensor("b1bc", [128, HD1], F32, kind="ExternalInput")
    blinbc = nc.dram_tensor("blinbc", [128, OUT], F32, kind="ExternalInput")
    srcidx = nc.dram_tensor("srcidx", [128, 8 * totT], I16, kind="ExternalInput")
    dstloc = nc.dram_tensor("dstloc", [128, totT], F32, kind="ExternalInput")
    vbias = nc.dram_tensor("vbias", [128, totT], F32, kind="ExternalInput")
    out_ext = nc.dram_tensor("out", [NPC, OUT], F32, kind="ExternalOutput")

    import contextlib

    with tile.TileContext(nc) as tc:
        with tc.tile_pool(name="dram", bufs=1, space="DRAM") as dram, \
             tc.tile_pool(name="constp", bufs=1) as constp, \
             tc.tile_pool(name="hold", bufs=1) as hold, \
             tc.tile_pool(name="work", bufs=2) as work, \
             (tc.For_i(0, reps, 1) if reps > 1 else contextlib.nullcontext()):

            fs_c = dram.tile([NPC, HD0], BF16)
            fd_c = dram.tile([NPC, HD0], BF16)
            res_c = dram.tile([NPC, HD0], F32)
            zes_c = dram.tile([NPC, 64], F32)
            fs1p_c = dram.tile([NPC, 64], F32)
            ASP = "Shared" if comms else "Local"
            fs_full = dram.tile([N, HD0], BF16, addr_space=ASP)
            zes_full = dram.tile([N, 64], F32, addr_space=ASP)
            fs1p_full = dram.tile([N, 64], F32, addr_space=ASP)
            cs_bounce = dram.tile([1, IN], F32)
            cs_sum = dram.tile([1, IN], F32, addr_space=ASP)

            ident = constp.tile([128, 128], F32)
            make_identity(nc, ident[:])
            iota_row_i = constp.tile([128, 128], I32)
            nc.gpsimd.iota(iota_row_i[:], pattern=[[1, 128]], channel_multiplier=0)
            iota_row = constp.tile([128, 128], F32)
            nc.vector.tensor_copy(out=iota_row[:], in_=iota_row_i[:])
            iota_col_i = constp.tile([128, 128], I32)
            nc.gpsimd.iota(iota_col_i[:], pattern=[[0, 128]], channel_multiplier=1)
            iota_col = constp.tile([128, 128], F32)
            nc.vector.tensor_copy(out=iota_col[:], in_=iota_col_i[:])
            ones128 = constp.tile([128, 1], F32)
            nc.vector.memset(ones128[:], 1.0)
            ones_row = constp.tile([1, 128], F32)
            nc.vector.memset(ones_row[:], 1.0)

            def load_const(name, dramt, shape, dt=F32):
                t = constp.tile(shape, dt, tag=name, name=name)
                nc.sync.dma_start(out=t[:], in_=dramt[:])
                return t
            a0b = load_const("a0b", a0bc, [128, HD0])
            a1b = load_const("a1b", a1bc, [128, HD1])
            a2sb = load_const("a2sb", a2sbc, [128, HD1])
            a2db = load_const("a2db", a2dbc, [128, HD1])
            b1b = load_const("b1b", b1bc, [128, HD1])
            blinb = load_const("blinb", blinbc, [128, OUT])
            dlocs = load_const("dlocs", dstloc, [128, totT])
            vbs = load_const("vbs", vbias, [128, totT])
            sidx = load_const("sidx", srcidx, [128, 8 * totT], I16)
            wlsb = load_const("wlsb", wlin, [2 * HD1, OUT])

            catT = hold.tile([64, NPC], F32)
            ed_t = [hold.tile([128, H], BF16, tag=f"ed{m}", name=f"ed{m}")
                    for m in range(WPC)]
            fd1_t = [hold.tile([128, HD1], F32, tag=f"fd1_{m}", name=f"fd1_{m}")
                     for m in range(WPC)]
            res1_t = [hold.tile([128, HD1], F32, tag=f"res1_{m}",
                                name=f"res1_{m}") for m in range(WPC)]

            ncol = [(j * 512, min(512, IN - j * 512))
                    for j in range((IN + 511) // 512)]

            # ============ Phases P, D0, z (hT alive) ============
            with tc.tile_pool(name="hpool", bufs=1) as hpool:
                hT = hpool.tile([128, K0T * NPC], BF16)

                with tc.tile_pool(name="pp", bufs=1) as pp, \
                     tc.tile_pool(name="psP", bufs=1, space="PSUM") as psP:
                    # pass 1: column sums (streamed feature tiles)
                    cs_sb = pp.tile([1, IN], F32, tag="cs_sb")
                    cpss = [psP.tile([1, 512], F32, tag=f"cs{j}", name=f"cs{j}",
                                     space="PSUM") for j in range(len(ncol))]
                    for m in range(WPC):
                        pr = _wrows(m)
                        ft = pp.tile([128, IN], F32, tag="fstream", name="ft",
                                     bufs=3)
                        nc.sync.dma_start(out=ft[:pr, :],
                                          in_=feat[m * 128:m * 128 + pr, :])
                        for j, (c0, cw) in enumerate(ncol):
                            nc.tensor.matmul(out=cpss[j][:1, :cw],
                                             lhsT=ones128[:pr, :],
                                             rhs=ft[:pr, c0:c0 + cw],
                                             start=(m == 0), stop=(m == WPC - 1))
                    for j, (c0, cw) in enumerate(ncol):
                        nc.scalar.copy(out=cs_sb[:, c0:c0 + cw],
                                       in_=cpss[j][:1, :cw])
                    nc.gpsimd.dma_start(out=cs_bounce[:], in_=cs_sb[:])
                    if comms:
                        nc.gpsimd.collective_compute(
                            "AllReduce", OP.add,
                            replica_groups=[list(range(NC))],
                            ins=[cs_bounce[:]], outs=[cs_sum[:]])
                    else:
                        nc.gpsimd.dma_start(out=cs_sum[:], in_=cs_bounce[:])
                    meanh = pp.tile([1, IN], F32, tag="meanh")
                    nc.sync.dma_start(out=meanh[:], in_=cs_sum[:])
                    nc.scalar.mul(out=meanh[:], in_=meanh[:], mul=0.5 / N)
                    meanb = pp.tile([128, IN], F32, tag="meanb")
                    for j, (c0, cw) in enumerate(ncol):
                        bps = psP.tile([128, 512], F32, tag="bps", name="bps",
                                       space="PSUM")
                        nc.tensor.matmul(out=bps[:, :cw], lhsT=ones_row[:, :],
                                         rhs=meanh[:, c0:c0 + cw],
                                         start=True, stop=True)
                        nc.scalar.copy(out=meanb[:, c0:c0 + cw], in_=bps[:, :cw])

                    # pass 2: impute + L1-normalize + transpose into hT (bf16)
                    for m in range(WPC):
                        pr = _wrows(m)
                        ft = pp.tile([128, KA], F32, tag="fstream2", name="ft",
                                     bufs=3)
                        nc.sync.dma_start(out=ft[:pr, 0:IN],
                                          in_=feat[m * 128:m * 128 + pr, :])
                        nc.vector.memset(ft[:, IN:KA], 1.0)
                        msk = pp.tile([128, IN], F32, tag="msk", name="msk",
                                      bufs=2)
                        nc.vector.tensor_scalar(out=msk[:pr, :],
                                                in0=ft[:pr, 0:IN],
                                                scalar1=0.0, scalar2=None,
                                                op0=OP.is_equal)
                        nc.vector.tensor_tensor(out=msk[:pr, :], in0=msk[:pr, :],
                                                in1=meanb[:pr, :], op=OP.mult)
                        nc.vector.tensor_tensor(out=ft[:pr, 0:IN],
                                                in0=ft[:pr, 0:IN],
                                                in1=msk[:pr, :], op=OP.add)
                        rs = work.tile([128, 1], F32, tag="rs")
                        nc.vector.tensor_reduce(out=rs[:pr, :],
                                                in_=ft[:pr, 0:IN],
                                                axis=AX.X, op=OP.add,
                                                apply_absolute_value=True)
                        nc.vector.tensor_scalar(out=rs[:pr, :], in0=rs[:pr, :],
                                                scalar1=1e-12, scalar2=None,
                                                op0=OP.max)
                        rinv = work.tile([128, 1], F32, tag="rinv")
                        nc.vector.reciprocal(out=rinv[:pr, :], in_=rs[:pr, :])
                        nc.vector.tensor_scalar(out=ft[:pr, 0:IN],
                                                in0=ft[:pr, 0:IN],
                                                scalar1=rinv[:pr, 0:1],
                                                scalar2=None, op0=OP.mult)
                        for k in range(K0T):
                            kw = min(128, KA - k * 128)
                            tps = psP.tile([128, 128], F32, tag="tps", name="tps",
                                           space="PSUM", bufs=2)
                            nc.tensor.transpose(out=tps[:kw, :pr],
                                                in_=ft[:pr, k * 128:k * 128 + kw],
                                                identity=ident[:pr, :pr])
                            nc.scalar.copy(
                                out=hT[:kw, k * NPC + m * 128:
                                       k * NPC + m * 128 + pr],
                                in_=tps[:kw, :pr])

                # ---------- D0 dense (bf16 x bf16 -> f32 psum) ----------
                with tc.tile_pool(name="dpool", bufs=1) as dpool, \
                     tc.tile_pool(name="psD", bufs=2, space="PSUM") as psD:

                    def dense0(wdram, dest, odt):
                        wkt = [dpool.tile([128, HD0], BF16, tag=f"wk{k}",
                                          name=f"wk{k}", bufs=2)
                               for k in range(K0T)]
                        for k in range(K0T):
                            kw = min(128, KA - k * 128)
                            nc.sync.dma_start(out=wkt[k][:kw, :],
                                              in_=wdram[k * 128:k * 128 + kw, :])
                        for m in range(WPC):
                            pr = _wrows(m)
                            osb = dpool.tile([128, HD0], odt, tag=f"d0o{odt}",
                                             name="d0o", bufs=2)
                            for j in range(2):
                                ops = psD.tile([128, 512], F32, tag="d0ps",
                                               name="d0ps", space="PSUM")
                                for k in range(K0T):
                                    kw = min(128, KA - k * 128)
                                    nc.tensor.matmul(
                                        out=ops[:pr, :],
                                        lhsT=hT[:kw, k * NPC + m * 128:
                                                k * NPC + m * 128 + pr],
                                        rhs=wkt[k][:kw, j * 512:(j + 1) * 512],
                                        start=(k == 0), stop=(k == K0T - 1))
                                nc.scalar.copy(
                                    out=osb[:pr, j * 512:(j + 1) * 512],
                                    in_=ops[:pr, :])
                            nc.sync.dma_start(out=dest[m * 128:m * 128 + pr, :],
                                              in_=osb[:pr, :])

                    dense0(wl0a, fs_c, BF16)
                    if comms:
                        nc.gpsimd.collective_compute(
                            "AllGather", OP.bypass,
                            replica_groups=[list(range(NC))],
                            ins=[fs_c[:]], outs=[fs_full[:]])
                    else:
                        for r in range(NC):
                            nc.sync.dma_start(
                                out=fs_full[r * NPC:(r + 1) * NPC, :],
                                in_=fs_c[:, :])
                    dense0(wr0a, fd_c, BF16)
                    dense0(wres0a, res_c, F32)

                    w2sb = dpool.tile([128, K0T * HD1], BF16, tag="w2sb")
                    for k in range(K0T):
                        kw = min(128, KA - k * 128)
                        nc.sync.dma_start(out=w2sb[:kw, k * HD1:(k + 1) * HD1],
                                          in_=w2p[k * 128:k * 128 + kw, :])
                    for m in range(WPC):
                        pr = _wrows(m)
                        zps = psD.tile([128, HD1], F32, tag="zps", name="zps",
                                       space="PSUM")
                        for k in range(K0T):
                            kw = min(128, KA - k * 128)
                            nc.tensor.matmul(
                                out=zps[:pr, :],
                                lhsT=hT[:kw, k * NPC + m * 128:
                                        k * NPC + m * 128 + pr],
                                rhs=w2sb[:kw, k * HD1:(k + 1) * HD1],
                                start=(k == 0), stop=(k == K0T - 1))
                        zsb = dpool.tile([128, 64], F32, tag="zsb", name="zsb",
                                         bufs=2)
                        nc.vector.memset(zsb[:], 0.0)
                        nc.scalar.copy(out=zsb[:pr, 0:HD1], in_=zps[:pr, :])
                        tmp = dpool.tile([128, HD1], F32, tag="ztmp", name="ztmp",
                                         bufs=2)
                        nc.vector.tensor_tensor(out=tmp[:pr, :],
                                                in0=zsb[:pr, 0:HD1],
                                                in1=a2sb[:pr, :], op=OP.mult)
                        nc.vector.tensor_reduce(
                            out=zsb[:pr, 32:36],
                            in_=tmp[:pr, :].rearrange("p (h d) -> p h d", h=H),
                            axis=AX.X, op=OP.add)
                        edf = dpool.tile([128, H], F32, tag="edf", name="edf",
                                         bufs=2)
                        nc.vector.tensor_tensor(out=tmp[:pr, :],
                                                in0=zsb[:pr, 0:HD1],
                                                in1=a2db[:pr, :], op=OP.mult)
                        nc.vector.tensor_reduce(
                            out=edf[:pr, :],
                            in_=tmp[:pr, :].rearrange("p (h d) -> p h d", h=H),
                            axis=AX.X, op=OP.add)
                        nc.vector.tensor_copy(out=ed_t[m][:pr, :],
                                              in_=edf[:pr, :])
                        nc.sync.dma_start(out=zes_c[m * 128:m * 128 + pr, :],
                                          in_=zsb[:pr, :])
                    if comms:
                        nc.gpsimd.collective_compute(
                            "AllGather", OP.bypass,
                            replica_groups=[list(range(NC))],
                            ins=[zes_c[:]], outs=[zes_full[:]])
                    else:
                        for r in range(NC):
                            nc.sync.dma_start(
                                out=zes_full[r * NPC:(r + 1) * NPC, :],
                                in_=zes_c[:, :])

            # ============ shared sel-matrix builder ============
            def build_sel(t, dl, pspool, e0mode):
                sel = work.tile([128, 128], F32, tag=f"sel{t}", name=f"sel{t}")
                nc.vector.tensor_tensor(out=sel[:],
                                        in0=dl.to_broadcast([128, 128]),
                                        in1=iota_row[:], op=OP.is_equal)
                dps = pspool.tile([128, 128], F32, tag="dps", name="dps",
                                  space="PSUM")
                nc.tensor.transpose(out=dps[:], in_=dl.to_broadcast([128, 128]),
                                    identity=ident[:])
                dlT = work.tile([128, 128], F32, tag="dlT")
                nc.scalar.copy(out=dlT[:], in_=dps[:])
                if e0mode:
                    selT16 = work.tile([128, 128], BF16, tag="selT16",
                                       name="selT16")
                    nc.vector.tensor_tensor(out=selT16[:], in0=iota_col[:],
                                            in1=dlT[:], op=OP.is_equal)
                    sel16 = work.tile([128, 128], BF16, tag=f"sel16_{t}",
                                      name=f"sel16_{t}")
                    nc.vector.tensor_copy(out=sel16[:], in_=sel[:])
                    return sel, sel16, selT16
                selT = work.tile([128, 128], F32, tag="selT", name="selT")
                nc.vector.tensor_tensor(out=selT[:], in0=iota_col[:], in1=dlT[:],
                                        op=OP.is_equal)
                return sel, selT

            # ============ Phase E0 + inner (h1T alive through D1) ============
            with tc.tile_pool(name="h1pool", bufs=1) as h1pool:
                h1T = h1pool.tile([128, K1T * NPC], BF16)
                with tc.tile_pool(name="psE0", bufs=1, space="PSUM") as psE0:
                    for w in range(WPC):
                        T = Ts[w]
                        nloc = _wrows(w)
                        co = toff[w]
                        fsg = work.tile([128, Tmax * HD0], BF16, tag="fsg")
                        nc.gpsimd.dma_gather(
                            out_ap=fsg[:].rearrange("p (t e) -> p t e",
                                                    t=Tmax)[:, :T, :],
                            in_ap=fs_full[:],
                            idxs_ap=sidx[:, 8 * co:8 * (co + T)],
                            num_idxs=T * 128, num_idxs_reg=T * 128,
                            elem_size=HD0)
                        zesg = work.tile([128, Tmax * 64], F32, tag="zesg")
                        nc.gpsimd.dma_gather(
                            out_ap=zesg[:].rearrange("p (t e) -> p t e",
                                                     t=Tmax)[:, :T, :],
                            in_ap=zes_full[:],
                            idxs_ap=sidx[:, 8 * co:8 * (co + T)],
                            num_idxs=T * 128, num_idxs_reg=T * 128,
                            elem_size=64)
                        fdw = work.tile([128, HD0], BF16, tag="fdw")
                        nc.sync.dma_start(out=fdw[:nloc, :],
                                          in_=fd_c[w * 128:w * 128 + nloc, :])
                        resw = work.tile([128, HD0], F32, tag="resw")
                        nc.sync.dma_start(out=resw[:nloc, :],
                                          in_=res_c[w * 128:w * 128 + nloc, :])

                        sels, sel16s, els, el2s = [], [], [], []
                        dn_ps = psE0.tile([128, H], F32, tag="dn", name="dn",
                                          space="PSUM")
                        dn2_ps = psE0.tile([128, H], F32, tag="dn2", name="dn2",
                                           space="PSUM")
                        for t in range(T):
                            dl = dlocs[:, co + t:co + t + 1]
                            vb = vbs[:, co + t:co + t + 1]
                            sel, sel16, selT16 = build_sel(t, dl, psE0, True)
                            sels.append(sel)
                            sel16s.append(sel16)
                            fdx = psE0.tile([128, HD0], F32, tag="mm1024",
                                            name="fdx", space="PSUM")
                            for j in range(2):
                                nc.tensor.matmul(
                                    out=fdx[:, j * 512:(j + 1) * 512],
                                    lhsT=selT16[:nloc, :],
                                    rhs=fdw[:nloc, j * 512:(j + 1) * 512],
                                    start=True, stop=True)
                            tt = work.tile([128, HD0], F32, tag="tt")
                            nc.vector.tensor_tensor(
                                out=tt[:], in0=fsg[:, t * HD0:(t + 1) * HD0],
                                in1=fdx[:], op=OP.add)
                            nc.scalar.activation(out=tt[:], in_=tt[:],
                                                 func=AF.Lrelu, alpha=0.2)
                            nc.vector.tensor_tensor(out=tt[:], in0=tt[:],
                                                    in1=a0b[:], op=OP.mult)
                            lg = work.tile([128, H], F32, tag="lg")
                            nc.vector.tensor_reduce(
                                out=lg[:],
                                in_=tt[:].rearrange("p (h d) -> p h d", h=H),
                                axis=AX.X, op=OP.add)
                            el = work.tile([128, H], F32, tag=f"el{t}",
                                           name=f"el{t}")
                            nc.scalar.activation(out=el[:], in_=lg[:],
                                                 func=AF.Exp, bias=vb)
                            els.append(el)
                            nc.tensor.matmul(out=dn_ps[:], lhsT=sel[:, :],
                                             rhs=el[:], start=(t == 0),
                                             stop=(t == T - 1))
                            edx = psE0.tile([128, HD1], F32, tag="m32",
                                            name="edx", space="PSUM")
                            nc.tensor.matmul(out=edx[:, 0:H],
                                             lhsT=selT16[:nloc, :],
                                             rhs=ed_t[w][:nloc, :],
                                             start=True, stop=True)
                            lg2 = work.tile([128, H], F32, tag="lg2")
                            nc.vector.tensor_tensor(
                                out=lg2[:],
                                in0=zesg[:, t * 64 + 32:t * 64 + 36],
                                in1=edx[:, 0:H], op=OP.add)
                            nc.scalar.activation(out=lg2[:], in_=lg2[:],
                                                 func=AF.Lrelu, alpha=0.2)
                            el2 = work.tile([128, H], F32, tag=f"el2_{t}",
                                            name=f"el2_{t}")
                            nc.scalar.activation(out=el2[:], in_=lg2[:],
                                                 func=AF.Exp, bias=vb)
                            el2s.append(el2)
                            nc.tensor.matmul(out=dn2_ps[:], lhsT=sel[:, :],
                                             rhs=el2[:], start=(t == 0),
                                             stop=(t == T - 1))

                        idn = work.tile([128, H], F32, tag="idn")
                        nc.vector.tensor_scalar(out=idn[:], in0=dn_ps[:],
                                                scalar1=1e-9, scalar2=None,
                                                op0=OP.max)
                        nc.vector.reciprocal(out=idn[:], in_=idn[:])
                        idn2 = work.tile([128, H], F32, tag="idn2")
                        nc.vector.tensor_scalar(out=idn2[:], in0=dn2_ps[:],
                                                scalar1=1e-9, scalar2=None,
                                                op0=OP.max)
                        nc.vector.reciprocal(out=idn2[:], in_=idn2[:])

                        o_ps = psE0.tile([128, HD0], F32, tag="mm1024",
                                         name="o_ps", space="PSUM")
                        oz_ps = psE0.tile([128, HD1], F32, tag="m32",
                                          name="oz_ps", space="PSUM")
                        for t in range(T):
                            for h in range(H):
                                nc.vector.tensor_scalar(
                                    out=fsg[:, t * HD0 + h * D0:
                                            t * HD0 + (h + 1) * D0],
                                    in0=fsg[:, t * HD0 + h * D0:
                                            t * HD0 + (h + 1) * D0],
                                    scalar1=els[t][:, h:h + 1], scalar2=None,
                                    op0=OP.mult)
                                nc.vector.tensor_scalar(
                                    out=zesg[:, t * 64 + h * D1:
                                             t * 64 + (h + 1) * D1],
                                    in0=zesg[:, t * 64 + h * D1:
                                             t * 64 + (h + 1) * D1],
                                    scalar1=el2s[t][:, h:h + 1], scalar2=None,
                                    op0=OP.mult)
                            for j in range(2):
                                nc.tensor.matmul(
                                    out=o_ps[:, j * 512:(j + 1) * 512],
                                    lhsT=sel16s[t][:, :],
                                    rhs=fsg[:, t * HD0 + j * 512:
                                            t * HD0 + (j + 1) * 512],
                                    start=(t == 0), stop=(t == T - 1))
                            nc.tensor.matmul(out=oz_ps[:], lhsT=sels[t][:, :],
                                             rhs=zesg[:, t * 64:t * 64 + HD1],
                                             start=(t == 0), stop=(t == T - 1))

                        ho = work.tile([128, HD0], F32, tag="ho")
                        for h in range(H):
                            nc.vector.tensor_scalar(
                                out=ho[:nloc, h * D0:(h + 1) * D0],
                                in0=o_ps[:nloc, h * D0:(h + 1) * D0],
                                scalar1=idn[:nloc, h:h + 1], scalar2=None,
                                op0=OP.mult)
                        nc.vector.tensor_tensor(out=ho[:nloc, :],
                                                in0=ho[:nloc, :],
                                                in1=resw[:nloc, :], op=OP.add)
                        nc.scalar.activation(out=ho[:nloc, :], in_=ho[:nloc, :],
                                             func=AF.Relu)
                        for k in range(K1T):
                            tps = psE0.tile([128, 128], F32, tag="tpsE",
                                            name="tpsE", space="PSUM")
                            nc.tensor.transpose(
                                out=tps[:, :nloc],
                                in_=ho[:nloc, k * 128:(k + 1) * 128],
                                identity=ident[:nloc, :nloc])
                            nc.scalar.copy(
                                out=h1T[:, k * NPC + w * 128:
                                        k * NPC + w * 128 + nloc],
                                in_=tps[:, :nloc])
                        ozs = work.tile([128, HD1], F32, tag="ozs")
                        for h in range(H):
                            nc.vector.tensor_scalar(
                                out=ozs[:nloc, h * D1:(h + 1) * D1],
                                in0=oz_ps[:nloc, h * D1:(h + 1) * D1],
                                scalar1=idn2[:nloc, h:h + 1], scalar2=None,
                                op0=OP.mult)
                        zt = psE0.tile([128, 128], F32, tag="tpsE", name="zt",
                                       space="PSUM")
                        nc.tensor.transpose(out=zt[:HD1, :nloc],
                                            in_=ozs[:nloc, :],
                                            identity=ident[:nloc, :nloc])
                        nc.scalar.copy(out=catT[0:HD1, w * 128:w * 128 + nloc],
                                       in_=zt[:HD1, :nloc])

                # ---------- D1 dense ----------
                with tc.tile_pool(name="d1pool", bufs=1) as d1pool, \
                     tc.tile_pool(name="psD1", bufs=2, space="PSUM") as psD1:
                    wk1 = {}
                    for nm, wd in (("l", wl1), ("r", wr1), ("res", wres1)):
                        t = d1pool.tile([128, K1T * HD1], BF16, tag=f"wk1{nm}",
                                        name=f"wk1{nm}")
                        for k in range(K1T):
                            nc.sync.dma_start(out=t[:, k * HD1:(k + 1) * HD1],
                                              in_=wd[k * 128:(k + 1) * 128, :])
                        wk1[nm] = t
                    for m in range(WPC):
                        pr = _wrows(m)
                        outs = {}
                        for nm in ("l", "r", "res"):
                            p1 = psD1.tile([128, HD1], F32, tag=f"d1{nm}",
                                           name=f"d1{nm}", space="PSUM")
                            for k in range(K1T):
                                nc.tensor.matmul(
                                    out=p1[:pr, :],
                                    lhsT=h1T[:, k * NPC + m * 128:
                                             k * NPC + m * 128 + pr],
                                    rhs=wk1[nm][:, k * HD1:(k + 1) * HD1],
                                    start=(k == 0), stop=(k == K1T - 1))
                            outs[nm] = p1
                        f1 = d1pool.tile([128, 64], F32, tag="f1", name="f1",
                                         bufs=2)
                        nc.vector.memset(f1[:], 0.0)
                        nc.scalar.copy(out=f1[:pr, 0:HD1], in_=outs["l"][:pr, :])
                        nc.sync.dma_start(out=fs1p_c[m * 128:m * 128 + pr, :],
                                          in_=f1[:pr, :])
                        nc.scalar.copy(out=fd1_t[m][:pr, :],
                                       in_=outs["r"][:pr, :])
                        nc.scalar.copy(out=res1_t[m][:pr, :],
                                       in_=outs["res"][:pr, :])
                    if comms:
                        nc.gpsimd.collective_compute(
                            "AllGather", OP.bypass,
                            replica_groups=[list(range(NC))],
                            ins=[fs1p_c[:]], outs=[fs1p_full[:]])
                    else:
                        for r in range(NC):
                            nc.sync.dma_start(
                                out=fs1p_full[r * NPC:(r + 1) * NPC, :],
                                in_=fs1p_c[:, :])

            # ============ Phase E1 (all f32) ============
            with tc.tile_pool(name="psE1", bufs=1, space="PSUM") as psE1:
                for w in range(WPC):
                    T = Ts[w]
                    nloc = _wrows(w)
                    co = toff[w]
                    f1g = work.tile([128, Tmax * 64], F32, tag="f1g")
                    nc.gpsimd.dma_gather(
                        out_ap=f1g[:].rearrange("p (t e) -> p t e",
                                                t=Tmax)[:, :T, :],
                        in_ap=fs1p_full[:],
                        idxs_ap=sidx[:, 8 * co:8 * (co + T)],
                        num_idxs=T * 128, num_idxs_reg=T * 128, elem_size=64)
                    sels, els = [], []
                    dn_ps = psE1.tile([128, H], F32, tag="dn3", name="dn3",
                                      space="PSUM")
                    for t in range(T):
                        dl = dlocs[:, co + t:co + t + 1]
                        vb = vbs[:, co + t:co + t + 1]
                        sel, selT = build_sel(t, dl, psE1, False)
                        sels.append(sel)
                        fdx = psE1.tile([128, HD1], F32, tag="m32b", name="fdx1",
                                        space="PSUM")
                        nc.tensor.matmul(out=fdx[:], lhsT=selT[:nloc, :],
                                         rhs=fd1_t[w][:nloc, :], start=True,
                                         stop=True)
                        tt = work.tile([128, HD1], F32, tag="tt1")
                        nc.vector.tensor_tensor(out=tt[:],
                                                in0=f1g[:, t * 64:t * 64 + HD1],
                                                in1=fdx[:], op=OP.add)
                        nc.scalar.activation(out=tt[:], in_=tt[:], func=AF.Lrelu,
                                             alpha=0.2)
                        nc.vector.tensor_tensor(out=tt[:], in0=tt[:], in1=a1b[:],
                                                op=OP.mult)
                        lg = work.tile([128, H], F32, tag="lg3")
                        nc.vector.tensor_reduce(
                            out=lg[:],
                            in_=tt[:].rearrange("p (h d) -> p h d", h=H),
                            axis=AX.X, op=OP.add)
                        el = work.tile([128, H], F32, tag=f"el3_{t}",
                                       name=f"el3_{t}")
                        nc.scalar.activation(out=el[:], in_=lg[:], func=AF.Exp,
                                             bias=vb)
                        els.append(el)
                        nc.tensor.matmul(out=dn_ps[:], lhsT=sel[:, :], rhs=el[:],
                                         start=(t == 0), stop=(t == T - 1))
                    idn = work.tile([128, H], F32, tag="idn3")
                    nc.vector.tensor_scalar(out=idn[:], in0=dn_ps[:],
                                            scalar1=1e-9, scalar2=None,
                                            op0=OP.max)
                    nc.vector.reciprocal(out=idn[:], in_=idn[:])
                    o_ps = psE1.tile([128, HD1], F32, tag="m32b", name="o_ps1",
                                     space="PSUM")
                    for t in range(T):
                        for h in range(H):
                            nc.vector.tensor_scalar(
                                out=f1g[:, t * 64 + h * D1:
                                        t * 64 + (h + 1) * D1],
                                in0=f1g[:, t * 64 + h * D1:
                                        t * 64 + (h + 1) * D1],
                                scalar1=els[t][:, h:h + 1], scalar2=None,
                                op0=OP.mult)
                        nc.tensor.matmul(out=o_ps[:], lhsT=sels[t][:, :],
                                         rhs=f1g[:, t * 64:t * 64 + HD1],
                                         start=(t == 0), stop=(t == T - 1))
                    oo = work.tile([128, HD1], F32, tag="oo")
                    for h in range(H):
                        nc.vector.tensor_scalar(
                            out=oo[:nloc, h * D1:(h + 1) * D1],
                            in0=o_ps[:nloc, h * D1:(h + 1) * D1],
                            scalar1=idn[:nloc, h:h + 1], scalar2=None,
                            op0=OP.mult)
                    nc.vector.tensor_tensor(out=oo[:nloc, :], in0=oo[:nloc, :],
                                            in1=res1_t[w][:nloc, :], op=OP.add)
                    nc.vector.tensor_tensor(out=oo[:nloc, :], in0=oo[:nloc, :],
                                            in1=b1b[:nloc, :], op=OP.add)
                    nc.scalar.activation(out=oo[:nloc, :], in_=oo[:nloc, :],
                                         func=AF.Relu)
                    tp = psE1.tile([128, 128], F32, tag="tp1", name="tp1",
                                   space="PSUM")
                    nc.tensor.transpose(out=tp[:HD1, :nloc], in_=oo[:nloc, :],
                                        identity=ident[:nloc, :nloc])
                    nc.scalar.copy(out=catT[HD1:2 * HD1, w * 128:w * 128 + nloc],
                                   in_=tp[:HD1, :nloc])

            # ============ Phase F ============
            with tc.tile_pool(name="psF", bufs=2, space="PSUM") as psF:
                for m in range(WPC):
                    pr = _wrows(m)
                    fp = psF.tile([128, OUT], F32, tag="fin", name="fin",
                                  space="PSUM")
                    nc.tensor.matmul(out=fp[:pr, :],
                                     lhsT=catT[:, m * 128:m * 128 + pr],
                                     rhs=wlsb[:], start=True, stop=True)
                    osb = work.tile([128, OUT], F32, tag="osb")
                    nc.vector.tensor_tensor(out=osb[:pr, :], in0=fp[:pr, :],
                                            in1=blinb[:pr, :], op=OP.add)
                    nc.sync.dma_start(out=out_ext[m * 128:m * 128 + pr, :],
                                      in_=osb[:pr, :])

    nc.compile()
    return nc


def _prep_edges(src, dst):
    order = np.argsort(dst, kind="stable")
    ss = src[order].astype(np.int64)
    ds = dst[order].astype(np.int64)
    cnt = np.zeros((NC, WPC), np.int64)
    bounds = {}
    for c in range(NC):
        for w in range(WPC):
            lo = c * NPC + w * 128
            hi = min(c * NPC + (w + 1) * 128, (c + 1) * NPC)
            e0 = np.searchsorted(ds, lo, side="left")
            e1 = np.searchsorted(ds, hi, side="left")
            cnt[c, w] = e1 - e0
            bounds[(c, w)] = (e0, e1)
    nws = [int(cnt[:, w].max()) for w in range(WPC)]
    Ts = [max(1, math.ceil(nv / 128)) for nv in nws]
    totT = sum(Ts)
    per_core = []
    for c in range(NC):
        sidx = np.zeros((128, 8 * totT), np.int16)
        dloc = np.zeros((128, totT), np.float32)
        vb = np.full((128, totT), NEG, np.float32)
        co = 0
        for w in range(WPC):
            T = Ts[w]
            e0, e1 = bounds[(c, w)]
            k = e1 - e0
            slots = T * 128
            s = np.zeros(slots, np.int16)
            d = np.zeros(slots, np.float32)
            v = np.full(slots, NEG, np.float32)
            s[:k] = ss[e0:e1]
            d[:k] = (ds[e0:e1] - (c * NPC + w * 128)).astype(np.float32)
            v[:k] = 0.0
            cols = s.reshape(8 * T, 16).T
            sidx[:, 8 * co:8 * (co + T)] = np.tile(cols, (8, 1))
            dloc[:, co:co + T] = d.reshape(T, 128).T
            vb[:, co:co + T] = v.reshape(T, 128).T
            co += T
        per_core.append((sidx, dloc, vb))
    return Ts, per_core


def kernel(features, src, dst, textMask, audioMask, videoMask, W2, a2,
           Wl0, Wr0, a0, Wres0, b0, Wl1, Wr1, a1, Wres1, b1, Wlin, blin):
    features = np.asarray(features, np.float32)
    src = np.asarray(src, np.int32)
    dst = np.asarray(dst, np.int32)

    Ts, per_core = _prep_edges(src, dst)
    key = tuple(Ts)
    if key not in _compiled:
        _compiled.clear()
        _compiled[key] = _build_program(Ts)
    nc = _compiled[key]

    maskSum = (np.asarray(textMask) + np.asarray(audioMask)
               + np.asarray(videoMask)).astype(np.float32)

    def aug(Wm, brow=None):
        o = np.zeros((KA, Wm.shape[1]), np.float32)
        o[:IN] = Wm * maskSum[:, None]
        if brow is not None:
            o[IN] = brow
        return o.astype(NPBF)

    w2flat = np.asarray(W2, np.float32).transpose(1, 0, 2).reshape(IN, HD1)
    shared = {
        "wl0a": aug(np.asarray(Wl0, np.float32)),
        "wr0a": aug(np.asarray(Wr0, np.float32)),
        "wres0a": aug(np.asarray(Wres0, np.float32),
                      np.asarray(b0, np.float32)),
        "w2p": aug(w2flat),
        "wl1": np.asarray(Wl1, np.float32).astype(NPBF),
        "wr1": np.asarray(Wr1, np.float32).astype(NPBF),
        "wres1": np.asarray(Wres1, np.float32).astype(NPBF),
        "wlin": np.asarray(Wlin, np.float32),
        "a0bc": np.tile(np.asarray(a0, np.float32).reshape(1, HD0), (128, 1)),
        "a1bc": np.tile(np.asarray(a1, np.float32).reshape(1, HD1), (128, 1)),
        "a2sbc": np.tile(np.asarray(a2, np.float32)[:, :D1].reshape(1, HD1),
                         (128, 1)),
        "a2dbc": np.tile(np.asarray(a2, np.float32)[:, D1:].reshape(1, HD1),
                         (128, 1)),
        "b1bc": np.tile(np.asarray(b1, np.float32).reshape(1, HD1), (128, 1)),
        "blinbc": np.tile(np.asarray(blin, np.float32).reshape(1, OUT),
                          (128, 1)),
    }
    in_maps = []
    for c in range(NC):
        sidx, dloc, vb = per_core[c]
        m = dict(shared)
        m["feat"] = np.ascontiguousarray(features[c * NPC:(c + 1) * NPC])
        m["srcidx"] = sidx
        m["dstloc"] = dloc
        m["vbias"] = vb
        in_maps.append(m)

    global _last_in_maps
    _last_in_maps = in_maps
    res = run_bass_kernel_spmd(nc, in_maps, list(range(NC)))
    global last_exec_ns
    last_exec_ns = getattr(res, "exec_time_ns", None)
    return np.concatenate(
        [np.asarray(res.results[c]["out"]) for c in range(NC)], axis=0)
